# revision 6
# baseline (speedup 1.0000x reference)
"""Trainium2 Bass kernel for nn_BoundaryPredictor1 (segment_reduce).

Contract: kernel(**inputs) takes the FULL unsharded inputs
(hidden (16,4096,512), attention_mask (16,4096), u (16,4096),
w1 (512,512), b1 (512,), w2 (512,1), b2 (1,)) and returns the same
pytree as the reference: (pooled, loss, k, n, shortened_attention_mask).

Strategy (pure data parallel, 2 examples per core on 8 cores):
  - MLP logits via PE matmuls in transposed activations layout
    (X.T built on-chip with PE transposes).
  - Boundary decisions in exact threshold form: hard = (logits+logistic > 0).
  - Segment-local cumulative sums via DVE tensor_tensor_scan with a
    multiplicative reset (keep = 1 - hard[t-1]); per-token counts via a
    running-max scan over boundary positions.
  - Per-segment mean rows divided on ACT during PSUM->SBUF evacuation and
    written with one indirect-scatter DMA per 128 tokens; non-boundary rows
    carry an out-of-bounds sentinel index and are skipped (bounds_check),
    so pre-zeroed output rows >= nb stay zero.
"""

import sys

if '/opt/trn_rl_repo' not in sys.path:
    sys.path.insert(0, '/opt/trn_rl_repo')

import numpy as np

B, L, D, H = 16, 4096, 512, 512
NCORES = 8
BC = B // NCORES          # examples per core
NCH = 8                   # 512-token chunks per example
CH = 512
ROWS = 8                  # [8, 512] scalar-pipeline layout
BIG = 1048576.0           # sentinel index (> 4095) for skipped rows
PRIOR = 0.25

_RUNNER = None


def _apply_patches():
    """This container's walrus accepts at most ONE sync-wait per
    instruction; spread Tile's waits over same-engine NoOp carriers."""
    import concourse.tile as tile
    from concourse import mybir
    from concourse.vector_clock import ScopedClock

    if getattr(tile.TileContext, '_waitsplit_patched', False):
        return

    def _drain_and_barrier(self, tick_clock, wait_clock):
        nop_inst = self.nc.sync.nop(nofuse=True)
        wait_clock.add_sem_waits(
            nop_inst.ins, ScopedClock({None: tick_clock.global_clock}))
        si = nop_inst.ins.sync_info
        waits = list(si.on_wait) if si is not None else []
        if len(waits) > 1:
            si.on_wait = waits[:1]
            for w in waits[1:]:
                extra = self.nc.sync.nop(nofuse=True)
                extra.ins.sync_info = mybir.SyncInfo(on_wait=[w], on_update=[])
        self.nc.sync.drain()
        self.nc.all_engine_barrier()
        assert self.sems is not None
        popped = self.nc._tile_sem_poison_stack.pop()
        assert popped is self._sem_poison
        self.nc.clear_and_free_semaphores(list(self.sems.allocated().values()))
        self.nc.all_engine_barrier()

    tile.TileContext._drain_and_barrier = _drain_and_barrier
    tile.TileContext._waitsplit_patched = True


def _split_multi_waits(nc):
    from concourse import mybir
    n = [0]

    def fresh():
        n[0] += 1
        return f"I-waitsplit-{n[0]}"

    for f in nc.m.functions:
        for blk in f.blocks:
            out = []
            changed = False
            for inst in blk.instructions:
                si = getattr(inst, 'sync_info', None)
                waits = list(si.on_wait) if si is not None and si.on_wait else []
                if len(waits) > 1:
                    changed = True
                    for w in waits[:-1]:
                        out.append(mybir.InstNoOp(
                            name=fresh(), engine=inst.engine, bass_nofuse=True,
                            sync_info=mybir.SyncInfo(on_wait=[w], on_update=[])))
                    si.on_wait = waits[-1:]
                out.append(inst)
            if changed:
                blk.instructions = out


def _build_nc():
    import concourse.bass as bass
    import concourse.tile as tile
    from concourse import mybir
    from concourse.masks import make_identity

    f32 = mybir.dt.float32
    i32 = mybir.dt.int32
    Alu = mybir.AluOpType
    Act = mybir.ActivationFunctionType

    nc = bass.Bass()

    hid = nc.dram_tensor("hidden", [BC, L, D], f32, kind="ExternalInput")
    msk = nc.dram_tensor("mask", [BC, ROWS, CH], f32, kind="ExternalInput")
    uin = nc.dram_tensor("u", [BC, ROWS, CH], f32, kind="ExternalInput")
    w1d = nc.dram_tensor("w1", [D, H], f32, kind="ExternalInput")
    b1d = nc.dram_tensor("b1", [H], f32, kind="ExternalInput")
    w2d = nc.dram_tensor("w2", [H, 1], f32, kind="ExternalInput")
    b2d = nc.dram_tensor("b2", [1], f32, kind="ExternalInput")

    pooled_out = [
        nc.dram_tensor(f"pooled{e}", [L, D], f32, kind="ExternalOutput")
        for e in range(BC)
    ]
    sam_out = nc.dram_tensor("sam", [BC, ROWS, CH], f32, kind="ExternalOutput")
    scal_out = nc.dram_tensor("scal", [1, 2 * BC], f32, kind="ExternalOutput")

    with tile.TileContext(nc) as tc:
        import contextlib
        with contextlib.ExitStack() as ctx:
            singles = ctx.enter_context(tc.tile_pool(name="singles", bufs=1))
            xt_pool = ctx.enter_context(tc.tile_pool(name="xt", bufs=1))
            work = ctx.enter_context(tc.tile_pool(name="work", bufs=1))
            ps_big = ctx.enter_context(
                tc.tile_pool(name="psb", bufs=6, space="PSUM"))
            ps_small = ctx.enter_context(
                tc.tile_pool(name="pss", bufs=2, space="PSUM"))

            def wtile(shape, dtype, tag, name, bufs=1):
                return work.tile(shape, dtype, tag=tag, name=name, bufs=bufs)

            def pbig(name):
                return ps_big.tile([128, CH], f32, space="PSUM", tag="big",
                                   name=name, bufs=6)

            def psmall(shape, name):
                return ps_small.tile(shape, f32, space="PSUM", tag="small",
                                     name=name, bufs=2)

            # ---- constants ----
            ident = singles.tile([128, 128], f32)
            make_identity(nc, ident)
            ones_row = singles.tile([1, 128], f32)
            nc.vector.memset(ones_row, 1.0)
            ones_col = singles.tile([8, 1], f32)
            nc.vector.memset(ones_col, 1.0)

            iota_i = singles.tile([ROWS, CH], i32)
            nc.gpsimd.iota(iota_i, pattern=[[1, CH]], base=0,
                           channel_multiplier=CH)
            iota_f = singles.tile([ROWS, CH], f32)
            nc.vector.tensor_copy(out=iota_f, in_=iota_i)

            w1_sb = []
            for dk in range(4):
                t = singles.tile([128, H], f32, tag=f"w1_{dk}",
                                 name=f"w1sb{dk}")
                nc.sync.dma_start(out=t, in_=w1d[dk * 128:(dk + 1) * 128, :])
                w1_sb.append(t)
            w2_sb = singles.tile([128, 4], f32)
            nc.sync.dma_start(
                out=w2_sb, in_=w2d.rearrange("(a p) o -> p (a o)", p=128))
            b1_sb = singles.tile([128, 4], f32)
            nc.sync.dma_start(
                out=b1_sb, in_=b1d.rearrange("(a p) -> p a", p=128))
            b2_sb = singles.tile([1, 1], f32)
            nc.sync.dma_start(out=b2_sb, in_=b2d.rearrange("o -> o ()"))

            scal_sb = singles.tile([1, 2 * BC], f32)
            bc_reg = nc.gpsimd.to_reg(L - 1)

            for e in range(BC):
                # ======== phase A: input-only scalar prep ========
                # [8,512] working-array tag chains (disjoint lifetimes):
                #  A: mask8 -> y8 -> cnt8 -> idxf8
                #  B: u8 -> logistic8 -> seg8
                #  C: lg -> lr8 -> sam8
                #  D: lg2 -> logits8 -> csum8 -> hs8
                #  E: e1 -> x8 -> pb8 -> keep8
                #  F: hard8        G: m8 -> recip8
                mask8 = wtile([ROWS, CH], f32, "sA", f"mask8_{e}")
                nc.sync.dma_start(out=mask8, in_=msk[e])
                u8 = wtile([ROWS, CH], f32, "sB", f"u8_{e}")
                nc.sync.dma_start(out=u8, in_=uin[e])

                red = wtile([8, 1], f32, "red", f"red_{e}", bufs=2)
                nc.vector.tensor_reduce(out=red, in_=mask8,
                                        axis=mybir.AxisListType.X, op=Alu.add)
                len_ps = psmall([1, 1], f"len_ps_{e}")
                nc.tensor.matmul(out=len_ps, lhsT=red, rhs=ones_col,
                                 start=True, stop=True)
                len_sb = wtile([1, 1], f32, "len_sb", f"len_sb_{e}", bufs=2)
                nc.vector.tensor_copy(out=len_sb, in_=len_ps)
                lcol_ps = psmall([8, 1], f"lcol_ps_{e}")
                nc.tensor.matmul(out=lcol_ps, lhsT=ones_row[0:1, 0:8],
                                 rhs=len_sb, start=True, stop=True)
                lm1_col = wtile([8, 1], f32, "lm1", f"lm1_{e}", bufs=2)
                nc.vector.tensor_scalar_add(out=lm1_col, in0=lcol_ps,
                                            scalar1=-1.0)

                lg = wtile([ROWS, CH], f32, "sC", f"lg_{e}")
                nc.scalar.activation(out=lg, in_=u8, func=Act.Ln)
                lg2 = wtile([ROWS, CH], f32, "sD", f"lg2_{e}")
                nc.scalar.activation(out=lg2, in_=u8, func=Act.Ln,
                                     scale=-1.0, bias=1.0)
                logistic8 = wtile([ROWS, CH], f32, "sB", f"logistic8_{e}")
                nc.vector.tensor_tensor(out=logistic8, in0=lg, in1=lg2,
                                        op=Alu.subtract)

                e1 = wtile([ROWS, CH], f32, "sE", f"e1_{e}")
                nc.vector.tensor_scalar(out=e1, in0=iota_f, scalar1=lm1_col,
                                        scalar2=None, op0=Alu.is_equal)
                lr8 = wtile([ROWS, CH], f32, "sC", f"lr8_{e}")
                nc.vector.scalar_tensor_tensor(
                    out=lr8, in0=iota_f, scalar=float(L - 1), in1=e1,
                    op0=Alu.is_lt, op1=Alu.mult)

                # ======== phase B: MLP over 8 chunks of 512 tokens ========
                xt = [xt_pool.tile([128, L], f32, tag=f"xt{j}",
                                   name=f"xt{j}_{e}") for j in range(4)]
                logits_strip = wtile([1, L], f32, "logits_strip",
                                     f"logits_strip_{e}")

                for c in range(NCH):
                    t0 = c * CH
                    xnat = []
                    for g in range(4):
                        xn = work.tile([128, D], f32, tag="xn",
                                       name=f"xn_{e}_{c}_{g}", bufs=6)
                        nc.sync.dma_start(
                            out=xn,
                            in_=hid[e, t0 + g * 128:t0 + (g + 1) * 128, :])
                        xnat.append(xn)
                    for j in range(4):
                        tr_ps = pbig(f"tr_ps_{e}_{c}_{j}")
                        for g in range(4):
                            nc.tensor.transpose(
                                out=tr_ps[:, g * 128:(g + 1) * 128],
                                in_=xnat[g][:, j * 128:(j + 1) * 128],
                                identity=ident)
                        nc.vector.tensor_copy(out=xt[j][:, t0:t0 + CH],
                                              in_=tr_ps)
                    h1r = []
                    for j in range(4):
                        h1_ps = pbig(f"h1_ps_{e}_{c}_{j}")
                        for dk in range(4):
                            nc.tensor.matmul(
                                out=h1_ps,
                                lhsT=w1_sb[dk][:, j * 128:(j + 1) * 128],
                                rhs=xt[dk][:, t0:t0 + CH],
                                start=(dk == 0), stop=(dk == 3))
                        hr = work.tile([128, CH], f32, tag="h1r",
                                       name=f"h1r_{e}_{c}_{j}", bufs=5)
                        nc.scalar.activation(out=hr, in_=h1_ps, func=Act.Relu,
                                             bias=b1_sb[:, j:j + 1], scale=1.0)
                        h1r.append(hr)
                    log_ps = psmall([1, CH], f"log_ps_{e}_{c}")
                    for j in range(4):
                        nc.tensor.matmul(out=log_ps, lhsT=w2_sb[:, j:j + 1],
                                         rhs=h1r[j],
                                         start=(j == 0), stop=(j == 3))
                    nc.scalar.activation(out=logits_strip[0:1, t0:t0 + CH],
                                         in_=log_ps, func=Act.Identity,
                                         bias=b2_sb[0:1, 0:1], scale=1.0)

                # ======== phase C: boundary decisions / indices ========
                logits8 = wtile([ROWS, CH], f32, "sD", f"logits8_{e}")
                nc.gpsimd.dma_start(out=logits8, in_=logits_strip[:])
                x8 = wtile([ROWS, CH], f32, "sE", f"x8_{e}")
                nc.vector.tensor_tensor(out=x8, in0=logits8, in1=logistic8,
                                        op=Alu.add)
                hard8 = wtile([ROWS, CH], f32, "sF", f"hard8_{e}")
                nc.vector.scalar_tensor_tensor(
                    out=hard8, in0=x8, scalar=0.0, in1=mask8,
                    op0=Alu.is_gt, op1=Alu.mult)
                nc.vector.tensor_tensor(out=hard8, in0=hard8, in1=lr8,
                                        op=Alu.max)

                csum8 = wtile([ROWS, CH], f32, "sD", f"csum8_{e}")
                nc.vector.tensor_tensor_scan(
                    out=csum8, data0=hard8, data1=hard8, initial=0.0,
                    op0=Alu.add, op1=Alu.bypass)

                def hier(t8, op, combine_op, nm):
                    tot_ps = psmall([1, 8], f"tot_ps_{nm}")
                    nc.tensor.transpose(out=tot_ps, in_=t8[:, CH - 1:CH],
                                        identity=ident[0:8, 0:8])
                    tot = wtile([1, 8], f32, "tot", f"tot_{nm}", bufs=2)
                    nc.vector.tensor_copy(out=tot, in_=tot_ps)
                    sh = wtile([1, 8], f32, "sh", f"sh_{nm}", bufs=2)
                    nc.vector.memset(sh[0:1, 0:1], 0.0)
                    nc.vector.tensor_copy(out=sh[0:1, 1:8], in_=tot[0:1, 0:7])
                    ex = wtile([1, 8], f32, "ex", f"ex_{nm}", bufs=2)
                    nc.vector.tensor_tensor_scan(
                        out=ex, data0=sh, data1=sh, initial=0.0,
                        op0=op, op1=Alu.bypass)
                    car_ps = psmall([8, 1], f"car_ps_{nm}")
                    nc.tensor.matmul(out=car_ps, lhsT=ex,
                                     rhs=ones_row[0:1, 0:1],
                                     start=True, stop=True)
                    car = wtile([8, 1], f32, "car", f"car_{nm}", bufs=2)
                    nc.vector.tensor_copy(out=car, in_=car_ps)
                    nc.vector.tensor_scalar(out=t8, in0=t8, scalar1=car,
                                            scalar2=None, op0=combine_op)
                    return tot

                tot_c = hier(csum8, Alu.add, Alu.add, f"c_{e}")

                nb_sb = wtile([1, 1], f32, "nb_sb", f"nb_sb_{e}", bufs=2)
                nc.vector.tensor_reduce(out=nb_sb, in_=tot_c,
                                        axis=mybir.AxisListType.X, op=Alu.add)
                nbc_ps = psmall([8, 1], f"nbc_ps_{e}")
                nc.tensor.matmul(out=nbc_ps, lhsT=ones_row[0:1, 0:8],
                                 rhs=nb_sb, start=True, stop=True)
                nb_col = wtile([8, 1], f32, "nb_col", f"nb_col_{e}", bufs=2)
                nc.vector.tensor_copy(out=nb_col, in_=nbc_ps)

                seg8 = wtile([ROWS, CH], f32, "sB", f"seg8_{e}")
                nc.vector.tensor_tensor(out=seg8, in0=csum8, in1=hard8,
                                        op=Alu.subtract)

                y8 = wtile([ROWS, CH], f32, "sA", f"y8_{e}")
                nc.vector.scalar_tensor_tensor(
                    out=y8, in0=iota_f, scalar=1.0, in1=hard8,
                    op0=Alu.add, op1=Alu.mult)
                m8 = wtile([ROWS, CH], f32, "sG", f"m8_{e}")
                nc.vector.tensor_tensor_scan(
                    out=m8, data0=y8, data1=y8, initial=0.0,
                    op0=Alu.max, op1=Alu.bypass)
                hier(m8, Alu.max, Alu.max, f"m_{e}")

                pb8 = wtile([ROWS, CH], f32, "sE", f"pb8_{e}")
                nc.vector.memset(pb8[:, 0:1], 0.0)
                nc.vector.tensor_copy(out=pb8[:, 1:CH], in_=m8[:, 0:CH - 1])
                nc.gpsimd.dma_start(out=pb8[1:8, 0:1], in_=m8[0:7, CH - 1:CH])
                cnt8 = wtile([ROWS, CH], f32, "sA", f"cnt8_{e}")
                nc.vector.scalar_tensor_tensor(
                    out=cnt8, in0=iota_f, scalar=1.0, in1=pb8,
                    op0=Alu.add, op1=Alu.subtract)
                recip8 = wtile([ROWS, CH], f32, "sG", f"recip8_{e}")
                nc.vector.reciprocal(out=recip8, in_=cnt8)

                hs8 = wtile([ROWS, CH], f32, "sD", f"hs8_{e}")
                nc.vector.memset(hs8[:, 0:1], 0.0)
                nc.vector.tensor_copy(out=hs8[:, 1:CH], in_=hard8[:, 0:CH - 1])
                nc.gpsimd.dma_start(out=hs8[1:8, 0:1],
                                    in_=hard8[0:7, CH - 1:CH])
                keep8 = wtile([ROWS, CH], f32, "sE", f"keep8_{e}")
                nc.vector.tensor_scalar(out=keep8, in0=hs8, scalar1=-1.0,
                                        scalar2=1.0, op0=Alu.mult, op1=Alu.add)
                keep_strip = wtile([1, L], f32, "keep_strip",
                                   f"keep_strip_{e}")
                nc.gpsimd.dma_start(out=keep_strip[:], in_=keep8[:])
                keep_bc = wtile([128, L], f32, "keep_bc", f"keep_bc_{e}")
                ks_ap = keep_strip[0:1, :]
                ks_rep = bass.AP(tensor=ks_ap.tensor, offset=ks_ap.offset,
                                 ap=[list(ks_ap.ap[0]), [0, 128],
                                     list(ks_ap.ap[1])])
                nc.gpsimd.dma_start(out=keep_bc, in_=ks_rep)

                idxf8 = wtile([ROWS, CH], f32, "sA", f"idxf8_{e}")
                nc.vector.scalar_tensor_tensor(
                    out=idxf8, in0=seg8, scalar=-BIG, in1=hard8,
                    op0=Alu.add, op1=Alu.mult)
                nc.vector.tensor_scalar_add(out=idxf8, in0=idxf8, scalar1=BIG)

                sam8 = wtile([ROWS, CH], f32, "sC", f"sam8_{e}")
                nc.vector.tensor_scalar(out=sam8, in0=iota_f, scalar1=nb_col,
                                        scalar2=None, op0=Alu.is_lt)
                nc.sync.dma_start(out=sam_out[e], in_=sam8)
                nc.vector.tensor_copy(out=scal_sb[0:1, e:e + 1], in_=nb_sb)
                nc.vector.tensor_copy(out=scal_sb[0:1, BC + e:BC + e + 1],
                                      in_=len_sb)

                idx_cols, rec_cols = [], []
                for k in range(4):
                    cp = psmall([128, 8], f"cp_{e}_{k}")
                    nc.tensor.transpose(out=cp,
                                        in_=idxf8[:, k * 128:(k + 1) * 128],
                                        identity=ident[0:8, 0:8])
                    cf = wtile([128, 8], f32, "idxcf", f"idxcf_{e}_{k}",
                               bufs=2)
                    nc.vector.tensor_copy(out=cf, in_=cp)
                    ci = wtile([128, 8], i32, "idxci", f"idxci_{e}_{k}",
                               bufs=5)
                    nc.vector.tensor_copy(out=ci, in_=cf)
                    idx_cols.append(ci)
                    rp = psmall([128, 8], f"rp_{e}_{k}")
                    nc.tensor.transpose(out=rp,
                                        in_=recip8[:, k * 128:(k + 1) * 128],
                                        identity=ident[0:8, 0:8])
                    rf = wtile([128, 8], f32, "reccf", f"reccf_{e}_{k}",
                               bufs=5)
                    nc.vector.tensor_copy(out=rf, in_=rp)
                    rec_cols.append(rf)

                # ======== phase D: segment scans, transpose back, scatter ====
                for j in range(4):
                    nc.vector.tensor_tensor_scan(
                        out=xt[j], data0=keep_bc, data1=xt[j], initial=0.0,
                        op0=Alu.mult, op1=Alu.add)
                for c in range(NCH):
                    t0 = c * CH
                    for g in range(4):
                        gn_ps = pbig(f"gn_ps_{e}_{c}_{g}")
                        for j in range(4):
                            nc.tensor.transpose(
                                out=gn_ps[:, j * 128:(j + 1) * 128],
                                in_=xt[j][:, t0 + g * 128:t0 + (g + 1) * 128],
                                identity=ident)
                        stage = work.tile([128, D], f32, tag="stage",
                                          name=f"stage_{e}_{c}_{g}", bufs=6)
                        nc.scalar.activation(out=stage, in_=gn_ps,
                                             func=Act.Copy,
                                             scale=rec_cols[g][:, c:c + 1])
                        nc.gpsimd.indirect_dma_start(
                            out=pooled_out[e][:],
                            out_offset=bass.IndirectOffsetOnAxis(
                                ap=idx_cols[g][:, c:c + 1], axis=0),
                            in_=stage[:], in_offset=None,
                            bounds_check=bc_reg, oob_is_err=False)

            nc.sync.dma_start(out=scal_out[:], in_=scal_sb)

    _split_multi_waits(nc)
    return nc


def _make_runner():
    """Build the Bass program once and wrap it in a cached jitted
    shard_map executor (mirrors bass2jax.run_bass_via_pjrt multi-core)."""
    import jax
    import jax.numpy as jnp  # noqa: F401
    from jax.sharding import Mesh, PartitionSpec
    try:
        from jax.experimental.shard_map import shard_map
    except Exception:
        from jax.shard_map import shard_map  # newer jax
    from concourse import bass2jax, mybir

    _apply_patches()
    nc = _build_nc()
    bass2jax.install_neuronx_cc_hook()

    part_name = (nc.partition_id_tensor.name
                 if nc.partition_id_tensor else None)
    in_names, out_names, out_avals, zero_shapes = [], [], [], []
    for alloc in nc.m.functions[0].allocations:
        if not isinstance(alloc, mybir.MemoryLocationSet):
            continue
        name = alloc.memorylocations[0].name
        if alloc.kind == "ExternalInput":
            if name != part_name:
                in_names.append(name)
        elif alloc.kind == "ExternalOutput":
            out_names.append(name)
            shape = tuple(alloc.tensor_shape)
            dtype = mybir.dt.np(alloc.dtype)
            out_avals.append(jax.core.ShapedArray(shape, dtype))
            zero_shapes.append((shape, dtype))
    n_params = len(in_names)
    all_in_names = in_names + out_names
    if part_name is not None:
        all_in_names = all_in_names + [part_name]
    donate = tuple(range(n_params, n_params + len(out_names)))

    def _body(*args):
        operands = list(args)
        if part_name is not None:
            operands.append(bass2jax.partition_id_tensor())
        outs = bass2jax._bass_exec_p.bind(
            *operands,
            out_avals=tuple(out_avals),
            in_names=tuple(all_in_names),
            out_names=tuple(out_names),
            lowering_input_output_aliases=(),
            sim_require_finite=False,
            sim_require_nnan=False,
            nc=nc,
        )
        return tuple(outs)

    devices = jax.devices()[:NCORES]
    mesh = Mesh(np.asarray(devices), ("core",))
    in_specs = (PartitionSpec("core"),) * (n_params + len(out_names))
    out_specs = (PartitionSpec("core"),) * len(out_names)
    sharded = jax.jit(
        shard_map(_body, mesh=mesh, in_specs=in_specs, out_specs=out_specs,
                  check_rep=False),
        donate_argnums=donate, keep_unused=True)

    def run(per_core_inputs):
        """per_core_inputs: list of NCORES dicts keyed by in_names."""
        concat_in = [
            np.concatenate([np.asarray(per_core_inputs[c][n])
                            for c in range(NCORES)], axis=0)
            for n in in_names
        ]
        concat_zeros = [
            np.zeros((NCORES * s[0],) + tuple(s[1:]), dt)
            for s, dt in zero_shapes
        ]
        out_arrs = sharded(*concat_in, *concat_zeros)
        res = []
        for c in range(NCORES):
            res.append({
                name: np.asarray(out_arrs[i]).reshape(
                    (NCORES,) + tuple(out_avals[i].shape))[c]
                for i, name in enumerate(out_names)
            })
        return res

    run.in_names = in_names
    run.sharded = sharded
    return run


def _get_runner():
    global _RUNNER
    if _RUNNER is None:
        _RUNNER = _make_runner()
    return _RUNNER


def _per_core_inputs(hidden, attention_mask, u, w1, b1, w2, b2):
    hidden = np.ascontiguousarray(np.asarray(hidden, np.float32))
    mask = np.ascontiguousarray(
        np.asarray(attention_mask, np.float32).reshape(B, ROWS, CH))
    uu = np.ascontiguousarray(np.asarray(u, np.float32).reshape(B, ROWS, CH))
    w1 = np.asarray(w1, np.float32)
    b1 = np.asarray(b1, np.float32)
    w2 = np.asarray(w2, np.float32)
    b2 = np.asarray(b2, np.float32)
    maps = []
    for i in range(NCORES):
        s = slice(i * BC, (i + 1) * BC)
        maps.append(dict(hidden=hidden[s], mask=mask[s], u=uu[s],
                         w1=w1, b1=b1, w2=w2, b2=b2))
    return maps


def kernel(hidden, attention_mask, u, w1, b1, w2, b2):
    run = _get_runner()
    maps = _per_core_inputs(hidden, attention_mask, u, w1, b1, w2, b2)
    results = run(maps)

    pooled = np.empty((B, L, D), np.float32)
    sam = np.empty((B, ROWS, CH), np.float32)
    k_total = 0.0
    n_total = 0.0
    for i in range(NCORES):
        r = results[i]
        for e in range(BC):
            pooled[i * BC + e] = r[f"pooled{e}"]
            sam[i * BC + e] = r["sam"][e]
            k_total += float(r["scal"][0, e])
            n_total += float(r["scal"][0, BC + e])
    sam = sam.reshape(B, L)

    k = np.float32(k_total)
    n = np.float32(n_total)
    from math import lgamma, log, log1p
    kk, nn = float(k), float(n)
    log_coef = lgamma(nn + 1.0) - lgamma(kk + 1.0) - lgamma(nn - kk + 1.0)
    loss = np.float32(-(log_coef + kk * log(PRIOR)
                        + (nn - kk) * log1p(-PRIOR)) / nn)
    return pooled, loss, k, n, sam


# revision 7
# speedup vs baseline: 148.3619x; 148.3619x over previous
"""Trainium2 Bass kernel for nn_BoundaryPredictor1 (segment_reduce).

Contract: kernel(**inputs) takes the FULL unsharded inputs
(hidden (16,4096,512), attention_mask (16,4096), u (16,4096),
w1 (512,512), b1 (512,), w2 (512,1), b2 (1,)) and returns the same
pytree as the reference: (pooled, loss, k, n, shortened_attention_mask).

Strategy (pure data parallel, 2 examples per core on 8 cores):
  - MLP logits via PE matmuls in transposed activations layout
    (X.T built on-chip with PE transposes).
  - Boundary decisions in exact threshold form: hard = (logits+logistic > 0).
  - Segment-local cumulative sums via DVE tensor_tensor_scan with a
    multiplicative reset (keep = 1 - hard[t-1]); per-token counts via a
    running-max scan over boundary positions.
  - Per-segment mean rows divided on ACT during PSUM->SBUF evacuation and
    written with one indirect-scatter DMA per 128 tokens; non-boundary rows
    carry an out-of-bounds sentinel index and are skipped (bounds_check),
    so pre-zeroed output rows >= nb stay zero.
"""

import sys

if '/opt/trn_rl_repo' not in sys.path:
    sys.path.insert(0, '/opt/trn_rl_repo')

import numpy as np

B, L, D, H = 16, 4096, 512, 512
NCORES = 8
BC = B // NCORES          # examples per core
NCH = 8                   # 512-token chunks per example
CH = 512
ROWS = 8                  # [8, 512] scalar-pipeline layout
BIG = 1048576.0           # sentinel index (> 4095) for skipped rows
PRIOR = 0.25

_RUNNER = None


def _apply_patches():
    """This container's walrus accepts at most ONE sync-wait per
    instruction; spread Tile's waits over same-engine NoOp carriers."""
    import concourse.tile as tile
    from concourse import mybir
    from concourse.vector_clock import ScopedClock

    if getattr(tile.TileContext, '_waitsplit_patched', False):
        return

    def _drain_and_barrier(self, tick_clock, wait_clock):
        nop_inst = self.nc.sync.nop(nofuse=True)
        wait_clock.add_sem_waits(
            nop_inst.ins, ScopedClock({None: tick_clock.global_clock}))
        si = nop_inst.ins.sync_info
        waits = list(si.on_wait) if si is not None else []
        if len(waits) > 1:
            si.on_wait = waits[:1]
            for w in waits[1:]:
                extra = self.nc.sync.nop(nofuse=True)
                extra.ins.sync_info = mybir.SyncInfo(on_wait=[w], on_update=[])
        self.nc.sync.drain()
        self.nc.all_engine_barrier()
        assert self.sems is not None
        popped = self.nc._tile_sem_poison_stack.pop()
        assert popped is self._sem_poison
        self.nc.clear_and_free_semaphores(list(self.sems.allocated().values()))
        self.nc.all_engine_barrier()

    tile.TileContext._drain_and_barrier = _drain_and_barrier
    tile.TileContext._waitsplit_patched = True


def _split_multi_waits(nc):
    from concourse import mybir
    n = [0]

    def fresh():
        n[0] += 1
        return f"I-waitsplit-{n[0]}"

    for f in nc.m.functions:
        for blk in f.blocks:
            out = []
            changed = False
            for inst in blk.instructions:
                si = getattr(inst, 'sync_info', None)
                waits = list(si.on_wait) if si is not None and si.on_wait else []
                if len(waits) > 1:
                    changed = True
                    for w in waits[:-1]:
                        out.append(mybir.InstNoOp(
                            name=fresh(), engine=inst.engine, bass_nofuse=True,
                            sync_info=mybir.SyncInfo(on_wait=[w], on_update=[])))
                    si.on_wait = waits[-1:]
                out.append(inst)
            if changed:
                blk.instructions = out


def _build_nc():
    import concourse.bass as bass
    import concourse.tile as tile
    from concourse import mybir
    from concourse.masks import make_identity

    f32 = mybir.dt.float32
    i32 = mybir.dt.int32
    Alu = mybir.AluOpType
    Act = mybir.ActivationFunctionType

    nc = bass.Bass()

    hid = nc.dram_tensor("hidden", [BC, L, D], f32, kind="ExternalInput")
    msk = nc.dram_tensor("mask", [BC, ROWS, CH], f32, kind="ExternalInput")
    uin = nc.dram_tensor("u", [BC, ROWS, CH], f32, kind="ExternalInput")
    w1d = nc.dram_tensor("w1", [D, H], f32, kind="ExternalInput")
    b1d = nc.dram_tensor("b1", [H], f32, kind="ExternalInput")
    w2d = nc.dram_tensor("w2", [H, 1], f32, kind="ExternalInput")
    b2d = nc.dram_tensor("b2", [1], f32, kind="ExternalInput")

    pooled_out = [
        nc.dram_tensor(f"pooled{e}", [L, D], f32, kind="ExternalOutput")
        for e in range(BC)
    ]
    sam_out = nc.dram_tensor("sam", [BC, ROWS, CH], f32, kind="ExternalOutput")
    scal_out = nc.dram_tensor("scal", [1, 2 * BC], f32, kind="ExternalOutput")

    with tile.TileContext(nc) as tc:
        import contextlib
        with contextlib.ExitStack() as ctx:
            singles = ctx.enter_context(tc.tile_pool(name="singles", bufs=1))
            xt_pool = ctx.enter_context(tc.tile_pool(name="xt", bufs=1))
            work = ctx.enter_context(tc.tile_pool(name="work", bufs=1))
            ps_big = ctx.enter_context(
                tc.tile_pool(name="psb", bufs=6, space="PSUM"))
            ps_small = ctx.enter_context(
                tc.tile_pool(name="pss", bufs=2, space="PSUM"))

            def wtile(shape, dtype, tag, name, bufs=1):
                return work.tile(shape, dtype, tag=tag, name=name, bufs=bufs)

            def pbig(name):
                return ps_big.tile([128, CH], f32, space="PSUM", tag="big",
                                   name=name, bufs=6)

            def psmall(shape, name):
                return ps_small.tile(shape, f32, space="PSUM", tag="small",
                                     name=name, bufs=2)

            # ---- constants ----
            ident = singles.tile([128, 128], f32)
            make_identity(nc, ident)
            ones_row = singles.tile([1, 128], f32)
            nc.vector.memset(ones_row, 1.0)
            ones_col = singles.tile([8, 1], f32)
            nc.vector.memset(ones_col, 1.0)

            iota_i = singles.tile([ROWS, CH], i32)
            nc.gpsimd.iota(iota_i, pattern=[[1, CH]], base=0,
                           channel_multiplier=CH)
            iota_f = singles.tile([ROWS, CH], f32)
            nc.vector.tensor_copy(out=iota_f, in_=iota_i)

            w1_sb = []
            for dk in range(4):
                t = singles.tile([128, H], f32, tag=f"w1_{dk}",
                                 name=f"w1sb{dk}")
                nc.sync.dma_start(out=t, in_=w1d[dk * 128:(dk + 1) * 128, :])
                w1_sb.append(t)
            w2_sb = singles.tile([128, 4], f32)
            nc.sync.dma_start(
                out=w2_sb, in_=w2d.rearrange("(a p) o -> p (a o)", p=128))
            b1_sb = singles.tile([128, 4], f32)
            nc.sync.dma_start(
                out=b1_sb, in_=b1d.rearrange("(a p) -> p a", p=128))
            b2_sb = singles.tile([1, 1], f32)
            nc.sync.dma_start(out=b2_sb, in_=b2d.rearrange("o -> o ()"))

            scal_sb = singles.tile([1, 2 * BC], f32)
            bc_reg = nc.gpsimd.to_reg(L - 1)

            for e in range(BC):
                # ======== phase A: input-only scalar prep ========
                # [8,512] working-array tag chains (disjoint lifetimes):
                #  A: mask8 -> y8 -> cnt8 -> idxf8
                #  B: u8 -> logistic8 -> seg8
                #  C: lg -> lr8 -> sam8
                #  D: lg2 -> logits8 -> csum8 -> hs8
                #  E: e1 -> x8 -> pb8 -> keep8
                #  F: hard8        G: m8 -> recip8
                mask8 = wtile([ROWS, CH], f32, "sA", f"mask8_{e}")
                nc.sync.dma_start(out=mask8, in_=msk[e])
                u8 = wtile([ROWS, CH], f32, "sB", f"u8_{e}")
                nc.sync.dma_start(out=u8, in_=uin[e])

                red = wtile([8, 1], f32, "red", f"red_{e}", bufs=2)
                nc.vector.tensor_reduce(out=red, in_=mask8,
                                        axis=mybir.AxisListType.X, op=Alu.add)
                len_ps = psmall([1, 1], f"len_ps_{e}")
                nc.tensor.matmul(out=len_ps, lhsT=red, rhs=ones_col,
                                 start=True, stop=True)
                len_sb = wtile([1, 1], f32, "len_sb", f"len_sb_{e}", bufs=2)
                nc.vector.tensor_copy(out=len_sb, in_=len_ps)
                lcol_ps = psmall([8, 1], f"lcol_ps_{e}")
                nc.tensor.matmul(out=lcol_ps, lhsT=ones_row[0:1, 0:8],
                                 rhs=len_sb, start=True, stop=True)
                lm1_col = wtile([8, 1], f32, "lm1", f"lm1_{e}", bufs=2)
                nc.vector.tensor_scalar_add(out=lm1_col, in0=lcol_ps,
                                            scalar1=-1.0)

                lg = wtile([ROWS, CH], f32, "sC", f"lg_{e}")
                nc.scalar.activation(out=lg, in_=u8, func=Act.Ln)
                lg2 = wtile([ROWS, CH], f32, "sD", f"lg2_{e}")
                nc.scalar.activation(out=lg2, in_=u8, func=Act.Ln,
                                     scale=-1.0, bias=1.0)
                logistic8 = wtile([ROWS, CH], f32, "sB", f"logistic8_{e}")
                nc.vector.tensor_tensor(out=logistic8, in0=lg, in1=lg2,
                                        op=Alu.subtract)

                e1 = wtile([ROWS, CH], f32, "sE", f"e1_{e}")
                nc.vector.tensor_scalar(out=e1, in0=iota_f, scalar1=lm1_col,
                                        scalar2=None, op0=Alu.is_equal)
                lr8 = wtile([ROWS, CH], f32, "sC", f"lr8_{e}")
                nc.vector.scalar_tensor_tensor(
                    out=lr8, in0=iota_f, scalar=float(L - 1), in1=e1,
                    op0=Alu.is_lt, op1=Alu.mult)

                # ======== phase B: MLP over 8 chunks of 512 tokens ========
                xt = [xt_pool.tile([128, L], f32, tag=f"xt{j}",
                                   name=f"xt{j}_{e}") for j in range(4)]
                logits_strip = wtile([1, L], f32, "logits_strip",
                                     f"logits_strip_{e}")

                for c in range(NCH):
                    t0 = c * CH
                    xnat = []
                    for g in range(4):
                        xn = work.tile([128, D], f32, tag="xn",
                                       name=f"xn_{e}_{c}_{g}", bufs=6)
                        nc.sync.dma_start(
                            out=xn,
                            in_=hid[e, t0 + g * 128:t0 + (g + 1) * 128, :])
                        xnat.append(xn)
                    for j in range(4):
                        tr_ps = pbig(f"tr_ps_{e}_{c}_{j}")
                        for g in range(4):
                            nc.tensor.transpose(
                                out=tr_ps[:, g * 128:(g + 1) * 128],
                                in_=xnat[g][:, j * 128:(j + 1) * 128],
                                identity=ident)
                        nc.vector.tensor_copy(out=xt[j][:, t0:t0 + CH],
                                              in_=tr_ps)
                    h1r = []
                    for j in range(4):
                        h1_ps = pbig(f"h1_ps_{e}_{c}_{j}")
                        for dk in range(4):
                            nc.tensor.matmul(
                                out=h1_ps,
                                lhsT=w1_sb[dk][:, j * 128:(j + 1) * 128],
                                rhs=xt[dk][:, t0:t0 + CH],
                                start=(dk == 0), stop=(dk == 3))
                        hr = work.tile([128, CH], f32, tag="h1r",
                                       name=f"h1r_{e}_{c}_{j}", bufs=5)
                        nc.scalar.activation(out=hr, in_=h1_ps, func=Act.Relu,
                                             bias=b1_sb[:, j:j + 1], scale=1.0)
                        h1r.append(hr)
                    log_ps = psmall([1, CH], f"log_ps_{e}_{c}")
                    for j in range(4):
                        nc.tensor.matmul(out=log_ps, lhsT=w2_sb[:, j:j + 1],
                                         rhs=h1r[j],
                                         start=(j == 0), stop=(j == 3))
                    nc.scalar.activation(out=logits_strip[0:1, t0:t0 + CH],
                                         in_=log_ps, func=Act.Identity,
                                         bias=b2_sb[0:1, 0:1], scale=1.0)

                # ======== phase C: boundary decisions / indices ========
                logits8 = wtile([ROWS, CH], f32, "sD", f"logits8_{e}")
                nc.gpsimd.dma_start(out=logits8, in_=logits_strip[:])
                x8 = wtile([ROWS, CH], f32, "sE", f"x8_{e}")
                nc.vector.tensor_tensor(out=x8, in0=logits8, in1=logistic8,
                                        op=Alu.add)
                hard8 = wtile([ROWS, CH], f32, "sF", f"hard8_{e}")
                nc.vector.scalar_tensor_tensor(
                    out=hard8, in0=x8, scalar=0.0, in1=mask8,
                    op0=Alu.is_gt, op1=Alu.mult)
                nc.vector.tensor_tensor(out=hard8, in0=hard8, in1=lr8,
                                        op=Alu.max)

                csum8 = wtile([ROWS, CH], f32, "sD", f"csum8_{e}")
                nc.vector.tensor_tensor_scan(
                    out=csum8, data0=hard8, data1=hard8, initial=0.0,
                    op0=Alu.add, op1=Alu.bypass)

                def hier(t8, op, combine_op, nm):
                    tot_ps = psmall([1, 8], f"tot_ps_{nm}")
                    nc.tensor.transpose(out=tot_ps, in_=t8[:, CH - 1:CH],
                                        identity=ident[0:8, 0:8])
                    tot = wtile([1, 8], f32, "tot", f"tot_{nm}", bufs=2)
                    nc.vector.tensor_copy(out=tot, in_=tot_ps)
                    sh = wtile([1, 8], f32, "sh", f"sh_{nm}", bufs=2)
                    nc.vector.memset(sh[0:1, 0:1], 0.0)
                    nc.vector.tensor_copy(out=sh[0:1, 1:8], in_=tot[0:1, 0:7])
                    ex = wtile([1, 8], f32, "ex", f"ex_{nm}", bufs=2)
                    nc.vector.tensor_tensor_scan(
                        out=ex, data0=sh, data1=sh, initial=0.0,
                        op0=op, op1=Alu.bypass)
                    car_ps = psmall([8, 1], f"car_ps_{nm}")
                    nc.tensor.matmul(out=car_ps, lhsT=ex,
                                     rhs=ones_row[0:1, 0:1],
                                     start=True, stop=True)
                    car = wtile([8, 1], f32, "car", f"car_{nm}", bufs=2)
                    nc.vector.tensor_copy(out=car, in_=car_ps)
                    nc.vector.tensor_scalar(out=t8, in0=t8, scalar1=car,
                                            scalar2=None, op0=combine_op)
                    return tot

                tot_c = hier(csum8, Alu.add, Alu.add, f"c_{e}")

                nb_sb = wtile([1, 1], f32, "nb_sb", f"nb_sb_{e}", bufs=2)
                nc.vector.tensor_reduce(out=nb_sb, in_=tot_c,
                                        axis=mybir.AxisListType.X, op=Alu.add)
                nbc_ps = psmall([8, 1], f"nbc_ps_{e}")
                nc.tensor.matmul(out=nbc_ps, lhsT=ones_row[0:1, 0:8],
                                 rhs=nb_sb, start=True, stop=True)
                nb_col = wtile([8, 1], f32, "nb_col", f"nb_col_{e}", bufs=2)
                nc.vector.tensor_copy(out=nb_col, in_=nbc_ps)

                seg8 = wtile([ROWS, CH], f32, "sB", f"seg8_{e}")
                nc.vector.tensor_tensor(out=seg8, in0=csum8, in1=hard8,
                                        op=Alu.subtract)

                y8 = wtile([ROWS, CH], f32, "sA", f"y8_{e}")
                nc.vector.scalar_tensor_tensor(
                    out=y8, in0=iota_f, scalar=1.0, in1=hard8,
                    op0=Alu.add, op1=Alu.mult)
                m8 = wtile([ROWS, CH], f32, "sG", f"m8_{e}")
                nc.vector.tensor_tensor_scan(
                    out=m8, data0=y8, data1=y8, initial=0.0,
                    op0=Alu.max, op1=Alu.bypass)
                hier(m8, Alu.max, Alu.max, f"m_{e}")

                pb8 = wtile([ROWS, CH], f32, "sE", f"pb8_{e}")
                nc.vector.memset(pb8[:, 0:1], 0.0)
                nc.vector.tensor_copy(out=pb8[:, 1:CH], in_=m8[:, 0:CH - 1])
                nc.gpsimd.dma_start(out=pb8[1:8, 0:1], in_=m8[0:7, CH - 1:CH])
                cnt8 = wtile([ROWS, CH], f32, "sA", f"cnt8_{e}")
                nc.vector.scalar_tensor_tensor(
                    out=cnt8, in0=iota_f, scalar=1.0, in1=pb8,
                    op0=Alu.add, op1=Alu.subtract)
                recip8 = wtile([ROWS, CH], f32, "sG", f"recip8_{e}")
                nc.vector.reciprocal(out=recip8, in_=cnt8)

                hs8 = wtile([ROWS, CH], f32, "sD", f"hs8_{e}")
                nc.vector.memset(hs8[:, 0:1], 0.0)
                nc.vector.tensor_copy(out=hs8[:, 1:CH], in_=hard8[:, 0:CH - 1])
                nc.gpsimd.dma_start(out=hs8[1:8, 0:1],
                                    in_=hard8[0:7, CH - 1:CH])
                keep8 = wtile([ROWS, CH], f32, "sE", f"keep8_{e}")
                nc.vector.tensor_scalar(out=keep8, in0=hs8, scalar1=-1.0,
                                        scalar2=1.0, op0=Alu.mult, op1=Alu.add)
                keep_strip = wtile([1, L], f32, "keep_strip",
                                   f"keep_strip_{e}")
                nc.gpsimd.dma_start(out=keep_strip[:], in_=keep8[:])
                keep_bc = wtile([128, L], f32, "keep_bc", f"keep_bc_{e}")
                ks_ap = keep_strip[0:1, :]
                ks_rep = bass.AP(tensor=ks_ap.tensor, offset=ks_ap.offset,
                                 ap=[list(ks_ap.ap[0]), [0, 128],
                                     list(ks_ap.ap[1])])
                nc.gpsimd.dma_start(out=keep_bc, in_=ks_rep)

                idxf8 = wtile([ROWS, CH], f32, "sA", f"idxf8_{e}")
                nc.vector.scalar_tensor_tensor(
                    out=idxf8, in0=seg8, scalar=-BIG, in1=hard8,
                    op0=Alu.add, op1=Alu.mult)
                nc.vector.tensor_scalar_add(out=idxf8, in0=idxf8, scalar1=BIG)

                sam8 = wtile([ROWS, CH], f32, "sC", f"sam8_{e}")
                nc.vector.tensor_scalar(out=sam8, in0=iota_f, scalar1=nb_col,
                                        scalar2=None, op0=Alu.is_lt)
                nc.sync.dma_start(out=sam_out[e], in_=sam8)
                nc.vector.tensor_copy(out=scal_sb[0:1, e:e + 1], in_=nb_sb)
                nc.vector.tensor_copy(out=scal_sb[0:1, BC + e:BC + e + 1],
                                      in_=len_sb)

                idx_cols, rec_cols = [], []
                for k in range(4):
                    cp = psmall([128, 8], f"cp_{e}_{k}")
                    nc.tensor.transpose(out=cp,
                                        in_=idxf8[:, k * 128:(k + 1) * 128],
                                        identity=ident[0:8, 0:8])
                    cf = wtile([128, 8], f32, "idxcf", f"idxcf_{e}_{k}",
                               bufs=2)
                    nc.vector.tensor_copy(out=cf, in_=cp)
                    ci = wtile([128, 8], i32, "idxci", f"idxci_{e}_{k}",
                               bufs=5)
                    nc.vector.tensor_copy(out=ci, in_=cf)
                    idx_cols.append(ci)
                    rp = psmall([128, 8], f"rp_{e}_{k}")
                    nc.tensor.transpose(out=rp,
                                        in_=recip8[:, k * 128:(k + 1) * 128],
                                        identity=ident[0:8, 0:8])
                    rf = wtile([128, 8], f32, "reccf", f"reccf_{e}_{k}",
                               bufs=5)
                    nc.vector.tensor_copy(out=rf, in_=rp)
                    rec_cols.append(rf)

                # ======== phase D: segment scans, transpose back, scatter ====
                for j in range(4):
                    nc.vector.tensor_tensor_scan(
                        out=xt[j], data0=keep_bc, data1=xt[j], initial=0.0,
                        op0=Alu.mult, op1=Alu.add)
                for c in range(NCH):
                    t0 = c * CH
                    for g in range(4):
                        gn_ps = pbig(f"gn_ps_{e}_{c}_{g}")
                        for j in range(4):
                            nc.tensor.transpose(
                                out=gn_ps[:, j * 128:(j + 1) * 128],
                                in_=xt[j][:, t0 + g * 128:t0 + (g + 1) * 128],
                                identity=ident)
                        stage = work.tile([128, D], f32, tag="stage",
                                          name=f"stage_{e}_{c}_{g}", bufs=6)
                        nc.scalar.activation(out=stage, in_=gn_ps,
                                             func=Act.Copy,
                                             scale=rec_cols[g][:, c:c + 1])
                        nc.gpsimd.indirect_dma_start(
                            out=pooled_out[e][:],
                            out_offset=bass.IndirectOffsetOnAxis(
                                ap=idx_cols[g][:, c:c + 1], axis=0),
                            in_=stage[:], in_offset=None,
                            bounds_check=bc_reg, oob_is_err=False)

            nc.sync.dma_start(out=scal_out[:], in_=scal_sb)

    _split_multi_waits(nc)
    return nc


def _make_runner():
    """Build the Bass program once and wrap it in a cached jitted
    shard_map executor (mirrors bass2jax.run_bass_via_pjrt multi-core)."""
    import jax
    import jax.numpy as jnp  # noqa: F401
    from jax.sharding import Mesh, PartitionSpec
    try:
        jax.config.update("jax_compilation_cache_dir", "/tmp/jax_cache")
        jax.config.update("jax_persistent_cache_min_entry_size_bytes", -1)
        jax.config.update("jax_persistent_cache_min_compile_time_secs", 0.0)
    except Exception:
        pass
    try:
        from jax.experimental.shard_map import shard_map
    except Exception:
        from jax.shard_map import shard_map  # newer jax
    from concourse import bass2jax, mybir

    _apply_patches()
    nc = _build_nc()
    bass2jax.install_neuronx_cc_hook()

    part_name = (nc.partition_id_tensor.name
                 if nc.partition_id_tensor else None)
    in_names, out_names, out_avals, zero_shapes = [], [], [], []
    for alloc in nc.m.functions[0].allocations:
        if not isinstance(alloc, mybir.MemoryLocationSet):
            continue
        name = alloc.memorylocations[0].name
        if alloc.kind == "ExternalInput":
            if name != part_name:
                in_names.append(name)
        elif alloc.kind == "ExternalOutput":
            out_names.append(name)
            shape = tuple(alloc.tensor_shape)
            dtype = mybir.dt.np(alloc.dtype)
            out_avals.append(jax.core.ShapedArray(shape, dtype))
            zero_shapes.append((shape, dtype))
    n_params = len(in_names)
    all_in_names = in_names + out_names
    if part_name is not None:
        all_in_names = all_in_names + [part_name]
    donate = tuple(range(n_params, n_params + len(out_names)))

    def _body(*args):
        operands = list(args)
        if part_name is not None:
            operands.append(bass2jax.partition_id_tensor())
        outs = bass2jax._bass_exec_p.bind(
            *operands,
            out_avals=tuple(out_avals),
            in_names=tuple(all_in_names),
            out_names=tuple(out_names),
            lowering_input_output_aliases=(),
            sim_require_finite=False,
            sim_require_nnan=False,
            nc=nc,
        )
        return tuple(outs)

    devices = jax.devices()[:NCORES]
    mesh = Mesh(np.asarray(devices), ("core",))
    in_specs = (PartitionSpec("core"),) * (n_params + len(out_names))
    out_specs = (PartitionSpec("core"),) * len(out_names)
    sharded = jax.jit(
        shard_map(_body, mesh=mesh, in_specs=in_specs, out_specs=out_specs,
                  check_rep=False),
        donate_argnums=donate, keep_unused=True)

    def run(per_core_inputs):
        """per_core_inputs: list of NCORES dicts keyed by in_names."""
        concat_in = [
            np.concatenate([np.asarray(per_core_inputs[c][n])
                            for c in range(NCORES)], axis=0)
            for n in in_names
        ]
        concat_zeros = [
            np.zeros((NCORES * s[0],) + tuple(s[1:]), dt)
            for s, dt in zero_shapes
        ]
        out_arrs = sharded(*concat_in, *concat_zeros)
        res = []
        for c in range(NCORES):
            res.append({
                name: np.asarray(out_arrs[i]).reshape(
                    (NCORES,) + tuple(out_avals[i].shape))[c]
                for i, name in enumerate(out_names)
            })
        return res

    def bench(per_core_inputs, iters=5):
        """Device-resident timing: inputs staged once, zeros created
        on-device, only the sharded execute is timed."""
        import time
        from jax.sharding import NamedSharding
        sh = NamedSharding(mesh, PartitionSpec("core"))
        dev_in = [
            jax.device_put(
                np.concatenate([np.asarray(per_core_inputs[c][n])
                                for c in range(NCORES)], axis=0), sh)
            for n in in_names
        ]
        jax.block_until_ready(dev_in)
        zero_makers = [
            jax.jit(
                (lambda s, dt: (lambda: jnp.zeros(s, dt)))(
                    (NCORES * s[0],) + tuple(s[1:]), dt),
                out_shardings=sh)
            for s, dt in zero_shapes
        ]
        times = []
        out = None
        for _ in range(iters):
            zeros = [zm() for zm in zero_makers]
            jax.block_until_ready(zeros)
            t0 = time.perf_counter()
            out = sharded(*dev_in, *zeros)
            jax.block_until_ready(out)
            times.append(time.perf_counter() - t0)
        return times, out

    run.in_names = in_names
    run.sharded = sharded
    run.bench = bench
    return run


def _get_runner():
    global _RUNNER
    if _RUNNER is None:
        _RUNNER = _make_runner()
    return _RUNNER


def _per_core_inputs(hidden, attention_mask, u, w1, b1, w2, b2):
    hidden = np.ascontiguousarray(np.asarray(hidden, np.float32))
    mask = np.ascontiguousarray(
        np.asarray(attention_mask, np.float32).reshape(B, ROWS, CH))
    uu = np.ascontiguousarray(np.asarray(u, np.float32).reshape(B, ROWS, CH))
    w1 = np.asarray(w1, np.float32)
    b1 = np.asarray(b1, np.float32)
    w2 = np.asarray(w2, np.float32)
    b2 = np.asarray(b2, np.float32)
    maps = []
    for i in range(NCORES):
        s = slice(i * BC, (i + 1) * BC)
        maps.append(dict(hidden=hidden[s], mask=mask[s], u=uu[s],
                         w1=w1, b1=b1, w2=w2, b2=b2))
    return maps


def kernel(hidden, attention_mask, u, w1, b1, w2, b2):
    run = _get_runner()
    maps = _per_core_inputs(hidden, attention_mask, u, w1, b1, w2, b2)
    results = run(maps)

    pooled = np.empty((B, L, D), np.float32)
    sam = np.empty((B, ROWS, CH), np.float32)
    k_total = 0.0
    n_total = 0.0
    for i in range(NCORES):
        r = results[i]
        for e in range(BC):
            pooled[i * BC + e] = r[f"pooled{e}"]
            sam[i * BC + e] = r["sam"][e]
            k_total += float(r["scal"][0, e])
            n_total += float(r["scal"][0, BC + e])
    sam = sam.reshape(B, L)

    k = np.float32(k_total)
    n = np.float32(n_total)
    from math import lgamma, log, log1p
    kk, nn = float(k), float(n)
    log_coef = lgamma(nn + 1.0) - lgamma(kk + 1.0) - lgamma(nn - kk + 1.0)
    loss = np.float32(-(log_coef + kk * log(PRIOR)
                        + (nn - kk) * log1p(-PRIOR)) / nn)
    return pooled, loss, k, n, sam
